# revision 23
# baseline (speedup 1.0000x reference)
"""Trainium2 Bass kernel for 2-layer edge-MLP GNN with segment-min aggregation.

Strategy (8 NeuronCores, SPMD, dst-bucket sharding -> no collectives):
- Core k owns nodes [12500k, 12500(k+1)). Within a core, nodes with deg>0
  are sorted by degree (desc) and paired: pair i = (node 2i, node 2i+1) ->
  (stream A = partitions 0:64, stream B = 64:128) of column-slot i.
- Slot i's width = max over cores of deg(A-node i) (sorted profiles are
  near-identical across cores). A node with fewer edges than the slot width
  duplicates one of its edges (min is idempotent). Slots are packed into
  1024-column blocks (PSUM msg tile = 2 banks); a slot never straddles a
  block boundary.
- Blocks have a uniform slot width (profile is monotone non-increasing),
  so the segment-min is exactly ONE vector.tensor_reduce(min) on a
  [128, n, d] view of each 1024-wide PSUM msg tile (amortizes the 125 ns
  PSUM access cost). agg column of a node = its slot ordinal.
- Edge MLP: MM1 (K=12, bf16 hi/lo-split inputs+weights), one ACT relu
  (+b1, fp32->bf16) per block over only the n*d real columns (trailing
  pad cols of h stay stale; their msg columns are never read by the
  reduce), MM2 = single bf16 matmul (numerics verified: rel ~1e-2 <
  2e-2 budget). ab2 folded into update-MLP bias. A b1-into-MM1 fold
  (b1_in_mm=True: two all-ones rhs rows carry b1 hi/lo, relu bias
  becomes const 0.0) is implemented and correctness-verified but
  DISABLED: 3-window interleaved A/Bs measured ratios 0.83/1.06/1.04 --
  no reproducible win, and the fold adds 17% rhs DMA.
- Update MLP: uw1 matmul in fp16 (bf16-rate, 10-bit mantissa), the 7
  u-relu PSUM drains split DVE/ACT (u_on_dve="split") to balance the two
  drain engines, u = relu(uW1.T agg + ub1') streamed out as f16 and the
  64->1 output projection (uW2, ub2) applied on host (px_host=True) --
  drops 7 ACT px passes + 14 PE matmuls per layer vs computing x2 on
  device. ab2 is folded into ub1' (constant shifts commute with min).
- Engine balance per layer: DVE carries the 100 segment-min reduces
  (~1.24 us per 1024-col PSUM pass, the roofline) + ~4 u-relus; ACT the
  100 relu passes + 3 u-relus. Measured on HW: concurrent DVE+ACT PSUM
  drains contend ~15-20% (mutual slowdown measured via single-op For_i
  microbenchmarks: reduce 1241 ns alone -> 1448 ns paired with a relu),
  which sets the ~140-146 us/layer quiet-device pace. Alternatives that
  measured WORSE or are impossible on TRN2 hardware (bench_dve.py):
  16-bit PSUM matmul output (TRN3+), Pool/GPSIMD reading PSUM (no port),
  tensor_tensor with two PSUM operands (walrus rejects), DVE 2x/4x modes
  for tensor_reduce (not engaged even on SBUF f16), DMA draining PSUM
  (no fabric route), pair-merged 4D reduces and 2048-wide ACT passes
  (<1% on HW), interleave_update (+4%: PSUM slot stealing stalls the
  edge pipeline more than the ~9 us tail it hides).
- const DMAs ordered by first use and fanned across the SP and
  Activation DGE queues; chunk 0 of rhs preloaded into a persistent
  buffer; last update block trimmed to its 106 real columns. Do NOT
  route DMAs via nc.gpsimd: the SWDGE ucode path adds ~250 s of compile
  time for ~1 us of startup.
- One compiled program, launched once per layer; host stages x[src] rows
  between launches (inter-layer gather + unpack are host-side).
"""

import math

import numpy as np
import ml_dtypes

import concourse.bass as bass
import concourse.bacc as bacc
import concourse.mybir as mybir
import concourse.tile as tile
from concourse.bass_utils import run_bass_kernel_spmd

F32 = mybir.dt.float32
F32R = mybir.dt.float32r
F16 = mybir.dt.float16
BF16 = mybir.dt.bfloat16

N_NODES = 100000
N_EDGES = 1600000
N_CORES = 8
NODES_PER_CORE = N_NODES // N_CORES
HID = 64
BLOCK = 1024  # msg/pre PSUM tile width (2 banks)
CHUNK = 8     # rhs tiles (of 512) per staging DMA


def _bf(a):
    return a.astype(ml_dtypes.bfloat16).astype(np.float32)


def _split_hi_lo(a):
    hi = _bf(a)
    return hi, a - hi


# ----------------------------------------------------------------------------
# Host-side layout construction (shared compiled structure across cores)
# ----------------------------------------------------------------------------

def build_layout(edge_index):
    src = np.asarray(edge_index[0], np.int64)
    dst = np.asarray(edge_index[1], np.int64)
    deg = np.bincount(dst, minlength=N_NODES)

    # CSR over edges by dst
    order = np.argsort(dst, kind="stable")
    starts = np.searchsorted(dst[order], np.arange(N_NODES))

    # per-core degree-desc sorted nonzero nodes, paired into streams A/B
    nodesA = []
    nodesB = []
    for k in range(N_CORES):
        nk = np.arange(k * NODES_PER_CORE, (k + 1) * NODES_PER_CORE)
        nz = nk[deg[nk] > 0]
        o = np.argsort(-deg[nz], kind="stable")
        snodes = nz[o]
        nodesA.append(snodes[0::2])
        nodesB.append(snodes[1::2])
    P = max(len(a) for a in nodesA)
    nA = np.full((N_CORES, P), -1, np.int64)
    nB = np.full((N_CORES, P), -1, np.int64)
    for k in range(N_CORES):
        nA[k, :len(nodesA[k])] = nodesA[k]
        nB[k, :len(nodesB[k])] = nodesB[k]
    degA = np.where(nA >= 0, deg[np.maximum(nA, 0)], 0)
    slotdeg = degA.max(axis=0)  # [P] width of each slot (desc-ish)
    assert slotdeg.min() >= 1

    # uniform-width blocks: slotdeg is non-increasing, so the width of a
    # block is its first slot's degree; every slot in the block is padded to
    # that width (duplicate edges; min is idempotent). Exactly one reduce
    # instruction per block.
    blk = np.zeros(P, np.int64)
    col0 = np.zeros(P, np.int64)
    width = np.zeros(P, np.int64)
    b, cur = 0, 0
    dblk = int(slotdeg[0])
    for i in range(P):
        if cur + dblk > BLOCK:
            b += 1
            cur = 0
            dblk = int(slotdeg[i])
        width[i] = dblk
        blk[i] = b
        col0[i] = b * BLOCK + cur
        cur += dblk
    NBLK = b + 1
    L = NBLK * BLOCK
    NT = L // 512

    # one run per block
    runs = [[] for _ in range(NBLK)]
    i = 0
    while i < P:
        j = i
        while j + 1 < P and blk[j + 1] == blk[i]:
            j += 1
        runs[blk[i]].append((0, int(j - i + 1), int(width[i]), int(i)))
        i = j + 1
    slotdeg = width

    # slot -> edge-id assignment, vectorized per (core, stream)
    tot = int(slotdeg.sum())
    slot_rep = np.repeat(np.arange(P), slotdeg)
    base = np.repeat(np.concatenate([[0], np.cumsum(slotdeg)[:-1]]), slotdeg)
    off = np.arange(tot) - base          # 0..slotdeg[i)-1 within slot
    pos = np.repeat(col0, slotdeg) + off  # global column

    slot_edge = np.zeros((N_CORES, 2, L), np.int64)
    node_pos = np.full((N_NODES, 2), -1, np.int64)
    for k in range(N_CORES):
        fill_edge = order[starts[nodesA[k][0]]]
        slot_edge[k, :, :] = fill_edge
        for s, nodes in ((0, nA[k]), (1, nB[k])):
            nd = nodes[slot_rep]
            valid = nd >= 0
            ndv = nd[valid]
            j = np.minimum(off[valid], deg[ndv] - 1)
            eids = order[starts[ndv] + j]
            slot_edge[k, s, pos[valid]] = eids
            real = nodes >= 0
            node_pos[nodes[real], 0] = s
            node_pos[nodes[real], 1] = np.arange(P)[real]

    C = P
    UB = (C + BLOCK - 1) // BLOCK
    C_pad = UB * BLOCK
    zero_nodes = np.where(deg == 0)[0]
    return dict(
        NBLK=NBLK, L=L, NT=NT, C=C, C_pad=C_pad, UB=UB, runs=runs,
        slot_edge=slot_edge, node_pos=node_pos, zero_nodes=zero_nodes,
        src=src, dst=dst,
    )


def build_rhs(layout, x_full, edge_attr, b1_in_mm=False):
    """Per-core rhs [12(+2), L] bf16; rows per stream s:
    [xhi, xhi, xlo, ehi, ehi, elo] at rows 6s..6s+5.
    With b1_in_mm, rows 12-13 are all-ones (carry b1 hi/lo through MM1)."""
    L = layout["L"]
    src = layout["src"]
    KR = 14 if b1_in_mm else 12
    rhs = np.zeros((N_CORES, KR, L), np.float32)
    for k in range(N_CORES):
        for s in range(2):
            eids = layout["slot_edge"][k, s]
            xv = x_full[src[eids]]
            ev = edge_attr[eids]
            xhi, xlo = _split_hi_lo(xv)
            ehi, elo = _split_hi_lo(ev)
            r0 = 6 * s
            rhs[k, r0 + 0] = xhi
            rhs[k, r0 + 1] = xhi
            rhs[k, r0 + 2] = xlo
            rhs[k, r0 + 3] = ehi
            rhs[k, r0 + 4] = ehi
            rhs[k, r0 + 5] = elo
        if b1_in_mm:
            rhs[k, 12] = 1.0
            rhs[k, 13] = 1.0
    return rhs.astype(ml_dtypes.bfloat16)


def build_weights(aW1, ab1, aW2, ab2, uW1, ub1, uW2, ub2, b1_in_mm=False):
    """Pack one layer's weights for the compiled program."""
    # MM1 lhsT [12(+1), 128] bf16: per scalar v rows [vh*wh, vh*wl, vl*wh]
    KR = 14 if b1_in_mm else 12
    w1 = np.zeros((KR, 128), np.float32)
    for s in range(2):
        c0 = 64 * s
        r0 = 6 * s
        for scalar_i in range(2):  # x then e
            w = aW1[scalar_i]  # [64]
            wh, wl = _split_hi_lo(w)
            w1[r0 + 3 * scalar_i + 0, c0:c0 + 64] = wh
            w1[r0 + 3 * scalar_i + 1, c0:c0 + 64] = wl
            w1[r0 + 3 * scalar_i + 2, c0:c0 + 64] = wh
    if b1_in_mm:
        bh, bl = _split_hi_lo(np.concatenate([ab1, ab1]))
        w1[12] = bh
        w1[13] = bl
    # MM2 lhsT blockdiag single bf16 [128, 128]
    w2 = np.zeros((128, 128), np.float32)
    for s in range(2):
        w2[64 * s:64 * s + 64, 64 * s:64 * s + 64] = aW2
    # biases
    b1vec = np.concatenate([ab1, ab1]).reshape(128, 1).astype(np.float32)
    # fold ab2 into ub1: ub1' = uW1.T @ ab2 + ub1
    ub1f = (uW1.T @ ab2 + ub1).astype(np.float32)
    ub1vec = np.concatenate([ub1f, ub1f]).reshape(128, 1).astype(np.float32)
    uw1blk = np.zeros((128, 128), np.float32)
    uw1blk[:64, :64] = uW1
    uw1blk[64:, 64:] = uW1
    uw2blk = np.zeros((128, 2), np.float32)
    uw2blk[:64, 0] = uW2[:, 0]
    uw2blk[64:, 1] = uW2[:, 0]
    ub2vec = np.array([[ub2[0]], [ub2[0]]], np.float32)
    return dict(
        w1=w1.astype(ml_dtypes.bfloat16),
        w2=w2.astype(ml_dtypes.bfloat16),
        b1vec=b1vec, ub1vec=ub1vec,
        uw1blk=uw1blk.astype(np.float16), uw2blk=uw2blk.astype(np.float16),
        ub2vec=ub2vec,
    )


# ----------------------------------------------------------------------------
# Bass program (compiled once; same structure for all cores and both layers)
# ----------------------------------------------------------------------------

def build_program(layout, bench_reps=1, skip=(), fp16_update=True,
                  hp_bufs=3, st_bufs=3, pre_bufs=2, msg_bufs=2,
                  interleave_update=False, upd_pools=("pre", "msg"),
                  u_on_dve=False, px_copy=False, px_host=True,
                  b1_in_mm=False):
    KR = 14 if b1_in_mm else 12
    skip = set(skip)
    AGG_DT = F16 if fp16_update else F32
    NBLK, L, NT = layout["NBLK"], layout["L"], layout["NT"]
    UB, C_pad = layout["UB"], layout["C_pad"]
    runs = layout["runs"]

    # block index at which each update-block's agg columns are complete
    slot_blk = []
    for bi, rr in enumerate(runs):
        for (c0, n, dd, ac0) in rr:
            slot_blk.extend([bi] * n)
    C = layout["C"]
    ready_at = {}
    for ui in range(UB):
        last_slot = min((ui + 1) * BLOCK, C) - 1
        ready_at.setdefault(slot_blk[last_slot], []).append(ui)

    nc = bacc.Bacc("TRN2", target_bir_lowering=False, debug=False,
                   num_devices=N_CORES)
    rhs_d = nc.dram_tensor("rhs", [KR, L], BF16, kind="ExternalInput")
    w1_d = nc.dram_tensor("w1", [KR, 128], BF16, kind="ExternalInput")
    w2_d = nc.dram_tensor("w2", [128, 128], BF16, kind="ExternalInput")
    b1_d = nc.dram_tensor("b1v", [128, 1], F32, kind="ExternalInput")
    ub1_d = nc.dram_tensor("ub1v", [128, 1], F32, kind="ExternalInput")
    uw1_d = nc.dram_tensor("uw1", [128, 128], AGG_DT, kind="ExternalInput")
    if px_host:
        u_d = nc.dram_tensor("uout", [128, C_pad], F16, kind="ExternalOutput")
    else:
        uw2_d = nc.dram_tensor("uw2", [128, 2], AGG_DT, kind="ExternalInput")
        ub2_d = nc.dram_tensor("ub2v", [2, 1], F32, kind="ExternalInput")
        x2_d = nc.dram_tensor("x2out", [2, C_pad], F32, kind="ExternalOutput")

    with tile.TileContext(nc) as tc:
        with (
            tc.tile_pool(name="const", bufs=1) as constp,
            tc.tile_pool(name="stage", bufs=st_bufs) as stagep,
            tc.tile_pool(name="hpool", bufs=hp_bufs) as hp,
            tc.tile_pool(name="aggp", bufs=1) as aggp,
            tc.tile_pool(name="upool", bufs=2) as up,
            tc.tile_pool(name="x2p", bufs=1) as x2p,
            tc.tile_pool(name="prep", bufs=pre_bufs, space="PSUM") as prep,
            tc.tile_pool(name="msgp", bufs=msg_bufs, space="PSUM") as msgp,
        ):
            # DMA order = first-use order: w1/b1 gate block 0, then the
            # first rhs chunk (persistent buffer; rhs is constant across
            # reps), then w2, then the update-MLP constants (first needed
            # ~130 us in).
            # fan the startup-critical DMAs across different DGE queues so
            # their descriptor generation overlaps (sync/scalar/vector/
            # gpsimd each own a queue)
            w1_t = constp.tile([KR, 128], BF16)
            nc.sync.dma_start(w1_t[:], w1_d[:, :])
            st0 = constp.tile([KR, CHUNK * 512], BF16)
            ct0 = min(CHUNK, NT)
            nc.scalar.dma_start(st0[:, :2 * 512], rhs_d[:, :2 * 512])
            b1_t = constp.tile([128, 1], F32)
            nc.scalar.dma_start(b1_t[:], b1_d[:, :])
            w2_t = constp.tile([128, 128], BF16)
            nc.scalar.dma_start(w2_t[:], w2_d[:, :])
            for p0 in range(2 * 512, ct0 * 512, 3 * 512):
                pw = min(3 * 512, ct0 * 512 - p0)
                nc.sync.dma_start(st0[:, p0:p0 + pw], rhs_d[:, p0:p0 + pw])
            uw1_t = constp.tile([128, 128], AGG_DT)
            nc.sync.dma_start(uw1_t[:], uw1_d[:, :])
            ub1_t = constp.tile([128, 1], F32)
            nc.sync.dma_start(ub1_t[:], ub1_d[:, :])
            if not px_host:
                uw2_t = constp.tile([128, 2], AGG_DT)
                nc.sync.dma_start(uw2_t[:], uw2_d[:, :])
                ub2_t = constp.tile([2, 1], F32)
                nc.sync.dma_start(ub2_t[:], ub2_d[:, :])

            agg_t = aggp.tile([128, C_pad], AGG_DT)
            if not px_host:
                x2_t = x2p.tile([2, C_pad], F32)

            import contextlib

            pool_by = {"pre": (prep, "pre"), "msg": (msgp, "msg")}

            def emit_update(ui):
                o = ui * BLOCK
                w = min(BLOCK, C - o)      # real columns in this ublock
                pup, putag = pool_by[upd_pools[0]]
                pxp, pxtag = pool_by[upd_pools[1]]
                pu = pup.tile([128, BLOCK], F32, tag=putag)
                for c0 in range(0, w, 512):
                    cw = min(512, w - c0)
                    nc.tensor.matmul(pu[:, c0:c0 + cw], uw1_t[:],
                                     agg_t[:, o + c0:o + c0 + cw],
                                     start=True, stop=True)
                u_t = up.tile([128, BLOCK], AGG_DT, tag="u")
                dve_here = (u_on_dve is True
                            or (u_on_dve == "split" and ui % 2 == 0))
                if dve_here:
                    nc.vector.tensor_scalar(
                        u_t[:, :w], pu[:, :w], ub1_t[:], 0.0,
                        op0=mybir.AluOpType.add, op1=mybir.AluOpType.max)
                else:
                    nc.scalar.activation(u_t[:, :w], pu[:, :w],
                                         mybir.ActivationFunctionType.Relu,
                                         bias=ub1_t[:], scale=1.0)
                if px_host:
                    nc.sync.dma_start(u_d[:, o:o + w], u_t[:, :w])
                    return
                px = pxp.tile([2, BLOCK], F32, tag=pxtag)
                for c0 in range(0, w, 512):
                    cw = min(512, w - c0)
                    nc.tensor.matmul(px[:, c0:c0 + cw], uw2_t[:],
                                     u_t[:, c0:c0 + cw],
                                     start=True, stop=True)
                if px_copy:
                    nc.scalar.activation(
                        x2_t[:, o:o + w], px[:, :w],
                        mybir.ActivationFunctionType.Copy,
                        bias=0.0, scale=1.0)
                else:
                    nc.scalar.activation(
                        x2_t[:, o:o + w], px[:, :w],
                        mybir.ActivationFunctionType.Identity,
                        bias=ub2_t[:], scale=1.0)
                nc.sync.dma_start(x2_d[:, o:o + w],
                                  x2_t[:, o:o + w])

            loop_cm = tc.For_i(0, bench_reps) if bench_reps > 1 \
                else contextlib.nullcontext()
            with loop_cm:
                emitted_ui = set()
                # ---- edge pipeline (software-pipelined by one block) ----
                h_hist = [None, None]  # h tiles of recent blocks
                st = None
                for b in range(NBLK + 1):
                    if b >= 1 and 'mm2' not in skip:
                        hprev = h_hist[(b - 1) % 2]
                        # msg cols beyond the block's n*d real columns are
                        # never read by the reduce -- trim the second chunk
                        (_, pn, pd, _), = runs[b - 1]
                        wprev = min(pn * pd, BLOCK)
                        msg = msgp.tile([128, BLOCK], F32, tag="msg")
                        nc.tensor.matmul(msg[:, 0:512], w2_t[:],
                                         hprev[:, 0:512],
                                         start=True, stop=True)
                        nc.tensor.matmul(msg[:, 512:wprev], w2_t[:],
                                         hprev[:, 512:wprev],
                                         start=True, stop=True)
                        for (c0, n, d, ac0) in ([] if 'reduce' in skip
                                                else runs[b - 1]):
                            nc.vector.tensor_reduce(
                                agg_t[:, ac0:ac0 + n],
                                msg[:, c0:c0 + n * d].rearrange(
                                    "p (n d) -> p n d", d=d),
                                axis=mybir.AxisListType.X,
                                op=mybir.AluOpType.min)
                        if (interleave_update and 'update' not in skip
                                and 'reduce' not in skip):
                            for ui in ready_at.get(b - 1, []):
                                if b - 1 < NBLK - 1:
                                    emit_update(ui)
                                    emitted_ui.add(ui)
                    if b < NBLK:
                        t0 = 2 * b
                        if t0 % CHUNK == 0:
                            c = t0 // CHUNK
                            if c == 0:
                                st = st0   # preloaded once; rhs constant
                            else:
                                ct = min(CHUNK, NT - c * CHUNK)
                                st = stagep.tile([KR, CHUNK * 512], BF16,
                                                 tag="st")
                                nc.sync.dma_start(
                                    st[:, :ct * 512],
                                    rhs_d[:, c * CHUNK * 512:
                                          (c * CHUNK + ct) * 512])
                        j0 = t0 % CHUNK
                        # pre cols beyond this block's n*d are never read by
                        # the trimmed relu -- trim MM1's second chunk too
                        (_, cn, cd, _), = runs[b]
                        wcur = min(cn * cd, BLOCK)
                        pre = prep.tile([128, BLOCK], F32, tag="pre")
                        nc.tensor.matmul(pre[:, 0:512], w1_t[:],
                                         st[:, j0 * 512:(j0 + 1) * 512],
                                         start=True, stop=True)
                        nc.tensor.matmul(pre[:, 512:wcur], w1_t[:],
                                         st[:, (j0 + 1) * 512:
                                             (j0 + 1) * 512 + wcur - 512],
                                         start=True, stop=True)
                    if b < NBLK and 'act' not in skip:
                        # only the n*d real columns feed the reduce; the
                        # trailing pad columns of h stay stale (finite or NaN
                        # -- their msg columns are never read by the reduce)
                        (_, nn, dd, _), = runs[b]
                        wb = nn * dd
                        h_t = hp.tile([128, BLOCK], BF16, tag="h")
                        nc.scalar.activation(h_t[:, :wb], pre[:, :wb],
                                             mybir.ActivationFunctionType.Relu,
                                             bias=(0.0 if b1_in_mm
                                                   else b1_t[:]), scale=1.0)
                        h_hist[b % 2] = h_t

                # ---- update MLP (tail; skipped per-ui if interleaved) ----
                for ui in range(UB if 'update' not in skip else 0):
                    if ui in emitted_ui:
                        continue
                    emit_update(ui)
    nc.compile()
    return nc


# build_program kwargs used by kernel(); override for A/B experiments.
# u_on_dve="split" balances the 7 u-relu PSUM drains across DVE (even
# ublocks) and ACT (odd), keeping the DVE (100 segment-min reduces, the
# roofline) and ACT (100 relu passes) queues near-equal.
KERNEL_KW = dict(u_on_dve="split", px_host=True, b1_in_mm=False)


def _update_zero_nodes(x_next, zero_nodes, uW1, ub1, uW2, ub2, ab2):
    if len(zero_nodes) == 0:
        return
    # agg = 0 (+ folded ab2): u = relu(uW1.T @ ab2 + ub1); x = uW2.T u + ub2
    u = np.maximum(uW1.T @ ab2 + ub1, 0.0)
    x_val = float(uW2[:, 0] @ u + ub2[0])
    x_next[zero_nodes] = x_val


def kernel(x, edge_attr, aW1, ab1, aW2, ab2, uW1, ub1, uW2, ub2, edge_index):
    x = np.asarray(x, np.float32)
    edge_attr = np.asarray(edge_attr, np.float32)
    edge_index = np.asarray(edge_index)
    aW1 = np.asarray(aW1, np.float32); ab1 = np.asarray(ab1, np.float32)
    aW2 = np.asarray(aW2, np.float32); ab2 = np.asarray(ab2, np.float32)
    uW1 = np.asarray(uW1, np.float32); ub1 = np.asarray(ub1, np.float32)
    uW2 = np.asarray(uW2, np.float32); ub2 = np.asarray(ub2, np.float32)

    layout = build_layout(edge_index)
    nc = build_program(layout, **KERNEL_KW)

    x_cur = x[:, 0].copy()
    ea = edge_attr[:, 0]
    node_pos = layout["node_pos"]
    mapped = node_pos[:, 0] >= 0
    core_of_node = np.arange(N_NODES) // NODES_PER_CORE

    bmm = bool(KERNEL_KW.get("b1_in_mm"))
    for l in range(2):
        wts = build_weights(aW1[l], ab1[l], aW2[l], ab2[l],
                            uW1[l], ub1[l], uW2[l], ub2[l], b1_in_mm=bmm)
        rhs = build_rhs(layout, x_cur, ea, b1_in_mm=bmm)
        in_maps = []
        for k in range(N_CORES):
            m = {"rhs": np.asarray(rhs[k]),
                 "w1": wts["w1"], "w2": wts["w2"],
                 "b1v": wts["b1vec"], "ub1v": wts["ub1vec"],
                 "uw1": wts["uw1blk"]}
            in_maps.append(m)
        res = run_bass_kernel_spmd(nc, in_maps, core_ids=list(range(N_CORES)),
                                   trace=False)
        x_next = np.zeros(N_NODES, np.float32)
        C_pad = layout["C_pad"]
        for k in range(N_CORES):
            # device returns u = relu(uW1.T agg + ub1') [128, C_pad] f16;
            # apply the 64->1 output projection (uW2, ub2) here
            u_k = np.asarray(res.results[k]["uout"], np.float32)
            x2sc = np.einsum("j,sjc->sc", uW2[l][:, 0],
                             u_k.reshape(2, 64, C_pad)) + ub2[l, 0]
            sel = mapped & (core_of_node == k)
            ids = np.where(sel)[0]
            x_next[ids] = x2sc[node_pos[ids, 0], node_pos[ids, 1]]
        _update_zero_nodes(x_next, layout["zero_nodes"],
                           uW1[l], ub1[l], uW2[l], ub2[l], ab2[l])
        x_cur = x_next

    return x_cur[:, None].astype(np.float32)

